# revision 1
# baseline (speedup 1.0000x reference)
"""Single-head attention layer on 8 NeuronCores, data-parallel over batch.

Per core (one batch): x [T, D] with T=2048, D=1024.
    q = x@Wq.T, k = x@Wk.T, v = x@Wv.T
    score = q@k.T / sqrt(T); attn = softmax(score); out = (attn@v)@Wo.T

Everything on-chip is kept feature-major (transposed), so no transposes are
ever needed on the device (the host pre-transposes x and the weights, and
re-transposes the output):
    qT[h,t] = wqT.T @ xT          kT[h,s] = wkT.T @ xT
    v[s,h]  = xT.T @ wvT          scoreT[s,t] = kT_slice.T @ qT
    expT    = exp(scoreT/sqrt(T))            (no max subtraction: |score/sqrt(T)|<~5)
    denom   = partition_all_reduce(sum_s expT)   (softmax denominator on DVE+GpSimd,
                                                  result broadcast on all partitions)
    oT[h,t] = v_slice.T @ expT;  oT *= 1/denom
    outT[o,t] = woT_slice.T @ oT

All matmul operands are bf16 (the host pre-rounds x and the weights to
bf16), so every matmul runs at 1 cycle/row with fast weight load;
accumulation is fp32 in PSUM and the softmax normalization math is fp32.
"""

import numpy as np

P = 128


def _build_attention(tc, aps, D, T, TB, CH):
    """Emit the per-core attention kernel into TileContext `tc`.

    aps: dict with DRAM APs xT[D,T], wqT/wkT/wvT[D,D] ([x,h]), woT[D,D] ([h,o]),
         outT[D,T] ([o,t]).
    TB: t-block size for the attention phase. CH: x-streaming chunk size.
    """
    from contextlib import ExitStack

    import concourse.mybir as mybir
    from concourse import bass_isa
    from concourse.bass import ts

    nc = tc.nc
    fp32 = mybir.dt.float32
    bf16 = mybir.dt.bfloat16
    Exp = mybir.ActivationFunctionType.Exp

    XO = D // P          # x (contraction) tiles
    HO = D // P          # h tiles
    SO = T // P          # s tiles
    NTB = T // TB        # t blocks
    NCH = T // CH        # x-stream chunks over t/s
    VH = min(512, D)     # v-proj h chunk
    NVH = D // VH
    SCALE = float(1.0 / np.sqrt(np.float32(T)))

    xT, wqT, wkT, wvT, woT, outT = (
        aps["xT"], aps["wqT"], aps["wkT"], aps["wvT"], aps["woT"], aps["outT"],
    )

    with ExitStack() as top:
        persist = top.enter_context(tc.tile_pool(name="persist", bufs=1))

        kT = persist.tile([P, HO, T], bf16, name="kT", tag="kT")
        qT = persist.tile([P, HO, T], bf16, name="qT", tag="qT")
        vsb = persist.tile([P, SO, D], bf16, name="vsb", tag="vsb")
        # Phase-2 pools that fit alongside phase 1 are allocated up front, so
        # the phase transition only gates the Wo weight loads.
        exp_pool = top.enter_context(tc.tile_pool(name="expp", bufs=SO))
        ot_pool = top.enter_context(tc.tile_pool(name="ot", bufs=HO + 2))
        out_pool = top.enter_context(tc.tile_pool(name="outp", bufs=4))
        rc_pool = top.enter_context(tc.tile_pool(name="rc", bufs=2))
        acc_pool = top.enter_context(tc.tile_pool(name="accp", bufs=1))
        ps_s = top.enter_context(tc.tile_pool(name="pss", bufs=2, space="PSUM"))

        # ---------------- phase 1: q/k/v projections (x streamed once) ------
        # Inputs arrive bf16 from the host; weight loads are interleaved with
        # the first chunk's matmuls so the PE starts as early as possible.
        w_pool = top.enter_context(tc.tile_pool(name="w", bufs=1))
        with ExitStack() as ph1:
            xs_pool = ph1.enter_context(tc.tile_pool(name="xs", bufs=2))
            ps1 = ph1.enter_context(tc.tile_pool(name="ps1", bufs=4, space="PSUM"))

            wq = [w_pool.tile([P, D], bf16, name=f"wq{x}", tag=f"wq{x}") for x in range(XO)]
            wk = [w_pool.tile([P, D], bf16, name=f"wk{x}", tag=f"wk{x}") for x in range(XO)]
            wv = [w_pool.tile([P, D], bf16, name=f"wv{x}", tag=f"wv{x}") for x in range(XO)]
            # Critical-path loads: spread the first weight matrix over both
            # the HWDGE and SWDGE queue sets so the PE can start sooner.
            xts0 = xs_pool.tile([P, XO, CH], bf16, name="xs0", tag="xs")
            for x in range(XO):
                nc.sync.dma_start(xts0[:, x, :], xT[ts(x, P), ts(0, CH)])
                nc.sync.dma_start(wq[x][:, :D // 2], wqT[ts(x, P), :D // 2])
                nc.gpsimd.dma_start(wq[x][:, D // 2:], wqT[ts(x, P), D // 2:])

            def proj_qk(dst, w, xts, i):
                for h in range(HO):
                    ps = ps1.tile([P, CH], fp32, name="ps_qk", tag="ps")
                    for x in range(XO):
                        nc.tensor.matmul(
                            ps[:], w[x][:, ts(h, P)], xts[:, x, :],
                            start=(x == 0), stop=(x == XO - 1),
                        )
                    nc.scalar.copy(dst[:, h, ts(i, CH)], ps[:])

            def proj_v(xts, i):
                for sl in range(CH // P):
                    s = i * (CH // P) + sl
                    for hc in range(NVH):
                        vps = ps1.tile([P, VH], fp32, name="vps", tag="ps")
                        for x in range(XO):
                            nc.tensor.matmul(
                                vps[:], xts[:, x, ts(sl, P)], wv[x][:, ts(hc, VH)],
                                start=(x == 0), stop=(x == XO - 1),
                            )
                        nc.vector.tensor_copy(vsb[:, s, ts(hc, VH)], vps[:])

            for i in range(NCH):
                if i == 0:
                    xts = xts0
                else:
                    xts = xs_pool.tile([P, XO, CH], bf16, name=f"xs{i}", tag="xs")
                    for x in range(XO):
                        nc.sync.dma_start(xts[:, x, :], xT[ts(x, P), ts(i, CH)])
                proj_qk(qT, wq, xts, i)
                if i == 0:
                    for x in range(XO):
                        nc.sync.dma_start(wk[x][:], wkT[ts(x, P), :])
                proj_qk(kT, wk, xts, i)
                if i == 0:
                    for x in range(XO):
                        nc.sync.dma_start(wv[x][:], wvT[ts(x, P), :])
                proj_v(xts, i)

            # Wo prefetch: reuse the Wq slots (free after the last q-proj
            # chunk), so the loads overlap the phase-1 tail.
            wo = [w_pool.tile([P, D], bf16, name=f"wo{h}", tag=f"wq{h}") for h in range(HO)]
            for h in range(HO):
                nc.sync.dma_start(wo[h][:], woT[ts(h, P), :])

        # ---------------- phase 2: attention + output projection ------------
        with ExitStack() as ph2:
            ps_o = ph2.enter_context(tc.tile_pool(name="pso", bufs=3, space="PSUM"))
            ps_w = ph2.enter_context(tc.tile_pool(name="psw", bufs=2, space="PSUM"))

            for tb in range(NTB):
                # scores + exp, s-tile at a time. The softmax denominator
                # accumulates on DVE in two halves; each half all-reduces
                # across partitions on the (otherwise idle) GpSimd as soon as
                # it is complete, so the reduce latency hides under the
                # remaining score matmuls.
                HALF = SO // 2
                accs = [
                    acc_pool.tile([P, TB], fp32, name=f"acc{j}", tag=f"acc{j}")
                    for j in range(2)
                ]
                dens = [
                    acc_pool.tile([P, TB], fp32, name=f"den{j}", tag=f"den{j}")
                    for j in range(2)
                ]
                exps = []
                for s in range(SO):
                    sps = ps_s.tile([P, TB], fp32, name="sps", tag="sps")
                    for h in range(HO):
                        nc.tensor.matmul(
                            sps[:], kT[:, h, ts(s, P)], qT[:, h, ts(tb, TB)],
                            start=(h == 0), stop=(h == HO - 1),
                        )
                    et = exp_pool.tile([P, TB], bf16, name=f"exp{s}", tag="exp")
                    nc.scalar.activation(et[:], sps[:], Exp, scale=SCALE)
                    exps.append(et)
                    j, sj = divmod(s, HALF)
                    if sj == 0:
                        nc.vector.tensor_copy(accs[j][:], et[:])
                    else:
                        nc.vector.tensor_add(accs[j][:], accs[j][:], et[:])
                    if sj == HALF - 1:
                        nc.gpsimd.partition_all_reduce(
                            dens[j][:], accs[j][:], channels=P,
                            reduce_op=bass_isa.ReduceOp.add,
                        )

                recip = rc_pool.tile([P, TB], fp32, name="recip", tag="recip")
                nc.vector.tensor_add(recip[:], dens[0][:], dens[1][:])
                nc.vector.reciprocal(recip[:], recip[:])

                # oT[h,:] = sum_s v[s,h-slice].T @ expT[s], then normalize
                ots = []
                for h in range(HO):
                    ops = ps_o.tile([P, TB], fp32, name="ops", tag="ops")
                    for s in range(SO):
                        nc.tensor.matmul(
                            ops[:], vsb[:, s, ts(h, P)], exps[s][:],
                            start=(s == 0), stop=(s == SO - 1),
                        )
                    ot = ot_pool.tile([P, TB], bf16, name=f"ot{h}", tag="ot")
                    nc.vector.tensor_mul(ot[:], ops[:], recip[:])
                    ots.append(ot)

                # output projection: outT[o,:] = sum_h woT[h,o-slice].T @ oT[h]
                for o in range(HO):
                    last = tb == NTB - 1 and o == HO - 1
                    # the final group runs in two column halves so its copy +
                    # store overlap the preceding matmuls instead of trailing
                    # the kernel
                    for c0, cw in ([(0, TB // 2), (TB // 2, TB // 2)] if last
                                   else [(0, TB)]):
                        wps = ps_w.tile([P, TB], fp32, name="wps", tag="wps")
                        for h in range(HO):
                            nc.tensor.matmul(
                                wps[:, :cw], wo[h][:, ts(o, P)],
                                ots[h][:, c0:c0 + cw],
                                start=(h == 0), stop=(h == HO - 1),
                            )
                        osb = out_pool.tile([P, TB], fp32, name="osb", tag="osb")
                        nc.scalar.copy(osb[:, :cw], wps[:, :cw])
                        nc.sync.dma_start(
                            outT[ts(o, P), tb * TB + c0:tb * TB + c0 + cw],
                            osb[:, :cw],
                        )


def build_bass(D=1024, T=2048, TB=512, CH=512):
    import concourse.mybir as mybir
    import concourse.tile as tile
    from concourse import bacc

    fp32 = mybir.dt.float32
    bf16 = mybir.dt.bfloat16
    nc = bacc.Bacc("TRN2", debug=False)
    aps = {
        "xT": nc.dram_tensor("xT", [D, T], bf16, kind="ExternalInput")[:],
        "wqT": nc.dram_tensor("wqT", [D, D], bf16, kind="ExternalInput")[:],
        "wkT": nc.dram_tensor("wkT", [D, D], bf16, kind="ExternalInput")[:],
        "wvT": nc.dram_tensor("wvT", [D, D], bf16, kind="ExternalInput")[:],
        "woT": nc.dram_tensor("woT", [D, D], bf16, kind="ExternalInput")[:],
        "outT": nc.dram_tensor("outT", [D, T], fp32, kind="ExternalOutput")[:],
    }
    with tile.TileContext(nc) as tc:
        _build_attention(tc, aps, D=D, T=T, TB=TB, CH=CH)
    nc.compile()
    return nc


def kernel(x, W_q, W_k, W_v, W_o):
    from concourse import bass_utils

    import ml_dtypes

    bf16 = ml_dtypes.bfloat16
    x = np.asarray(x, dtype=np.float32)
    B = x.shape[0]
    wqT = np.ascontiguousarray(np.asarray(W_q, np.float32).T.astype(bf16))
    wkT = np.ascontiguousarray(np.asarray(W_k, np.float32).T.astype(bf16))
    wvT = np.ascontiguousarray(np.asarray(W_v, np.float32).T.astype(bf16))
    woT = np.ascontiguousarray(np.asarray(W_o, np.float32).T.astype(bf16))

    in_maps = [
        {
            "xT": np.ascontiguousarray(x[b].T.astype(bf16)),
            "wqT": wqT,
            "wkT": wkT,
            "wvT": wvT,
            "woT": woT,
        }
        for b in range(B)
    ]

    nc = build_bass()
    res = bass_utils.run_bass_kernel_spmd(nc, in_maps, core_ids=list(range(B)))
    out = np.stack([res.results[b]["outT"].T for b in range(B)])
    return np.ascontiguousarray(out.astype(np.float32))



# revision 2
# speedup vs baseline: 1.4948x; 1.4948x over previous
"""Single-head attention on 8 NeuronCores, data-parallel over batch.

Per core (one batch item): x [T, D] with T=2048, D=1024.
    q = x@Wq.T, k = x@Wk.T, v = x@Wv.T
    score = q@k.T / sqrt(T); attn = softmax(score); out = (attn@v)@Wo.T

Weight folding (host-side, fp32): the kernel never materializes q, k or
the output projection. With M = Wq.T@Wk and N = (Wo@Wv).T:
    score = (x@M) @ x.T          (q/k projections fold into one)
    out   = attn @ (x@N)         (Wo folds into the v projection)
Per-core work drops from 17.2 to 12.9 GMAC, and the score matmul's
stationary operand is the already-resident input x.

On-chip layout is feature-major (transposed) so no device transposes:
    zT[h,t]   = M.T @ xT                   (z = x@M)
    v'[s,h]   = xT.T @ N                   (v' = x@N)
    scoreT[s,t] = xT_slice.T @ zT
    expT      = exp(scoreT/sqrt(T))        (no max subtraction: exp <= ~55)
    denom     = partition_all_reduce(sum_s expT)  (two halves, hidden
                                                   under the score matmuls)
    oT[h,t]   = v'_slice.T @ expT;  outT = oT * (1/denom)

All matmul operands are bf16 (host pre-rounds), accumulation fp32 in
PSUM. Stationary tiles are loaded once and reused for 2-4 matmuls of
512 moving columns, so LDWEIGHTS overhead is ~7% instead of ~25%.
"""

import numpy as np

P = 128


def _build_attention(tc, aps, D, T):
    """Emit the per-core attention kernel into TileContext `tc`.

    aps: DRAM APs xT[D,T], mT[D,D] ([x,h] for z=x@M), nT[D,D] ([x,h] for
         v'=x@N), outT[D,T] ([h,t]).
    """
    from contextlib import ExitStack

    import concourse.mybir as mybir
    from concourse import bass_isa
    from concourse.bass import ts

    nc = tc.nc
    fp32 = mybir.dt.float32
    bf16 = mybir.dt.bfloat16
    Exp = mybir.ActivationFunctionType.Exp

    XO = D // P          # feature (contraction) tiles: 8
    HO = D // P          # h tiles: 8
    SO = T // P          # s tiles: 16
    TC = 512             # moving-column chunk (= one PSUM bank of fp32)
    NTC = T // TC        # 4
    HALF = SO // 2
    SCALE = float(1.0 / np.sqrt(np.float32(T)))

    xT, mT, nT, outT = aps["xT"], aps["mT"], aps["nT"], aps["outT"]

    with ExitStack() as top:
        persist = top.enter_context(tc.tile_pool(name="persist", bufs=1))
        xsb = persist.tile([P, XO, T], bf16, name="xsb", tag="xsb")
        zT = persist.tile([P, HO, T], bf16, name="zT", tag="zT")
        vsb = persist.tile([P, SO, D], bf16, name="vsb", tag="vsb")
        exp_pool = top.enter_context(tc.tile_pool(name="expp", bufs=SO))
        # One PSUM pool for every phase: 2 rotating [P, T] fp32 buffers
        # (4 banks each = all 8 banks), so phase transitions need no new
        # allocation and pipelining depth is uniform.
        ps = top.enter_context(tc.tile_pool(name="ps", bufs=2, space="PSUM"))

        # ---------------- phase 1: v' and z projections ---------------------
        # Both consume xT; v' runs first (stationary xT s-tiles, moving N
        # weights) while M streams in behind the first loads.
        with ExitStack() as ph1:
            w_pool = ph1.enter_context(tc.tile_pool(name="w", bufs=1))
            wn = [w_pool.tile([P, D], bf16, name=f"wn{x}", tag=f"wn{x}")
                  for x in range(XO)]
            wm = [w_pool.tile([P, D], bf16, name=f"wm{x}", tag=f"wm{x}")
                  for x in range(XO)]

            # Interleave the critical first loads across both DMA queue
            # sets: N-weights and the first t-chunk of x come first so the
            # v' matmuls can start, M afterwards (z-proj needs it ~60us in).
            for x in range(XO):
                eng = nc.sync if x % 2 == 0 else nc.gpsimd
                eng.dma_start(wn[x][:], nT[ts(x, P), :])
                eng.dma_start(xsb[:, x, ts(0, TC)], xT[ts(x, P), ts(0, TC)])
            for x in range(XO):
                eng = nc.sync if x % 2 == 0 else nc.gpsimd
                eng.dma_start(wm[x][:], mT[ts(x, P), :])
            for tcc in range(1, NTC):
                for x in range(XO):
                    eng = nc.sync if x % 2 == 0 else nc.gpsimd
                    eng.dma_start(xsb[:, x, ts(tcc, TC)],
                                  xT[ts(x, P), ts(tcc, TC)])

            # v'[s,h] = sum_x xT[x,s].T @ N[x,h]; one weight load per (s,x),
            # two 512-col matmuls per load.
            for s in range(SO):
                vps = ps.tile([P, T], fp32, name=f"vps{s}", tag="ps")
                for x in range(XO):
                    lhsT = xsb[:, x, ts(s, P)]
                    for c in range(D // TC):
                        nc.tensor.matmul(
                            vps[:, ts(c, TC)], lhsT, wn[x][:, ts(c, TC)],
                            start=(x == 0), stop=(x == XO - 1),
                        )
                nc.vector.tensor_copy(vsb[:, s, :], vps[:, :D])

            # zT[h,t] = sum_x M[x,h].T @ xT[x,t]; one weight load per (h,x),
            # four 512-col matmuls per load.
            for h in range(HO):
                zps = ps.tile([P, T], fp32, name=f"zps{h}", tag="ps")
                for x in range(XO):
                    lhsT = wm[x][:, ts(h, P)]
                    for tcc in range(NTC):
                        nc.tensor.matmul(
                            zps[:, ts(tcc, TC)], lhsT, xsb[:, x, ts(tcc, TC)],
                            start=(x == 0), stop=(x == XO - 1),
                        )
                nc.scalar.copy(zT[:, h, :], zps[:])

        # ---------------- phase 2: scores + softmax + A@v' ------------------
        with ExitStack() as ph2:
            red = ph2.enter_context(tc.tile_pool(name="red", bufs=1))
            accs = [red.tile([P, T], fp32, name=f"acc{j}", tag=f"acc{j}")
                    for j in range(2)]
            dens = [red.tile([P, T], fp32, name=f"den{j}", tag=f"den{j}")
                    for j in range(2)]

            # scoreT[s,t] = sum_h xT[h,s].T @ zT[h,t]. Stationary is the
            # resident xT; h=7 is emitted last so the final zT copy hides
            # under the h=0..6 matmuls. exp on Scalar; the softmax
            # denominator accumulates on DVE in two halves, each half
            # all-reduced across partitions on GpSimd as soon as it
            # completes so the reduce hides under the remaining matmuls.
            exps = []
            for s in range(SO):
                sps = ps.tile([P, T], fp32, name=f"sps{s}", tag="ps")
                for h in range(HO):
                    lhsT = xsb[:, h, ts(s, P)]
                    for tcc in range(NTC):
                        nc.tensor.matmul(
                            sps[:, ts(tcc, TC)], lhsT, zT[:, h, ts(tcc, TC)],
                            start=(h == 0), stop=(h == HO - 1),
                        )
                et = exp_pool.tile([P, T], bf16, name=f"exp{s}", tag="exp")
                nc.scalar.activation(et[:], sps[:], Exp, scale=SCALE)
                exps.append(et)
                j, sj = divmod(s, HALF)
                if sj == 0:
                    nc.vector.tensor_copy(accs[j][:], et[:])
                else:
                    nc.vector.tensor_add(accs[j][:], accs[j][:], et[:])
                if sj == HALF - 1:
                    nc.gpsimd.partition_all_reduce(
                        dens[j][:], accs[j][:], channels=P,
                        reduce_op=bass_isa.ReduceOp.add,
                    )

            # 1/denom: acc0's buffer is dead once its all-reduce is done.
            nc.vector.tensor_add(dens[0][:], dens[0][:], dens[1][:])
            rc = red.tile([P, T], fp32, name="rc", tag="acc0")
            nc.vector.reciprocal(rc[:], dens[0][:])

            # oT[h,t] = sum_s v'[s,h].T @ expT[s]; one weight load per
            # (h,s), four 512-col matmuls per load. Output tiles normalize
            # on DVE into the dead acc/den buffers and DMA out. The last
            # h-tile accumulates into two PSUM tiles (t-halves) so its
            # normalize+store overlaps the preceding matmuls.
            osb_tags = ["acc1", "den1", "den0"]
            n_osb = 0
            for h in range(HO):
                last = h == HO - 1
                if last:
                    opsA = ps.tile([P, T], fp32, name="opsA", tag="ps")
                    opsB = ps.tile([P, T], fp32, name="opsB", tag="ps")
                else:
                    opsA = ps.tile([P, T], fp32, name=f"ops{h}", tag="ps")
                    opsB = opsA
                for s in range(SO):
                    lhsT = vsb[:, s, ts(h, P)]
                    for tcc in range(NTC):
                        dst = opsA if tcc < 2 else opsB
                        nc.tensor.matmul(
                            dst[:, ts(tcc, TC)], lhsT, exps[s][:, ts(tcc, TC)],
                            start=(s == 0), stop=(s == SO - 1),
                        )
                halves = [(opsA, 0, T // 2), (opsB, T // 2, T // 2)] if last \
                    else [(opsA, 0, T)]
                for src, c0, cw in halves:
                    osb = red.tile([P, T], fp32, name=f"osb{n_osb}",
                                   tag=osb_tags[n_osb % 3])
                    n_osb += 1
                    nc.vector.tensor_mul(
                        osb[:, c0:c0 + cw], src[:, c0:c0 + cw], rc[:, c0:c0 + cw]
                    )
                    eng = nc.sync if h % 2 == 0 else nc.gpsimd
                    eng.dma_start(outT[ts(h, P), c0:c0 + cw], osb[:, c0:c0 + cw])


def build_bass(D=1024, T=2048):
    import concourse.mybir as mybir
    import concourse.tile as tile
    from concourse import bacc

    fp32 = mybir.dt.float32
    bf16 = mybir.dt.bfloat16
    nc = bacc.Bacc("TRN2", debug=False)
    aps = {
        "xT": nc.dram_tensor("xT", [D, T], bf16, kind="ExternalInput")[:],
        "mT": nc.dram_tensor("mT", [D, D], bf16, kind="ExternalInput")[:],
        "nT": nc.dram_tensor("nT", [D, D], bf16, kind="ExternalInput")[:],
        "outT": nc.dram_tensor("outT", [D, T], fp32, kind="ExternalOutput")[:],
    }
    with tile.TileContext(nc) as tc:
        _build_attention(tc, aps, D=D, T=T)
    nc.compile()
    return nc


def prepare_in_maps(x, W_q, W_k, W_v, W_o):
    """Host-side weight folding + per-core input maps (bf16, transposed)."""
    import ml_dtypes

    bf16 = ml_dtypes.bfloat16
    x = np.asarray(x, dtype=np.float32)
    Wq = np.asarray(W_q, np.float32)
    Wk = np.asarray(W_k, np.float32)
    Wv = np.asarray(W_v, np.float32)
    Wo = np.asarray(W_o, np.float32)
    mT = np.ascontiguousarray((Wq.T @ Wk).astype(bf16))
    nT = np.ascontiguousarray((Wo @ Wv).T.astype(bf16))
    return [
        {
            "xT": np.ascontiguousarray(x[b].T.astype(bf16)),
            "mT": mT,
            "nT": nT,
        }
        for b in range(x.shape[0])
    ]


def kernel(x, W_q, W_k, W_v, W_o):
    from concourse import bass_utils

    in_maps = prepare_in_maps(x, W_q, W_k, W_v, W_o)
    B = len(in_maps)
    nc = build_bass()
    res = bass_utils.run_bass_kernel_spmd(nc, in_maps, core_ids=list(range(B)))
    out = np.stack([res.results[b]["outT"].T for b in range(B)])
    return np.ascontiguousarray(out.astype(np.float32))
